# revision 20
# baseline (speedup 1.0000x reference)
"""Trainium2 kernel for AdaptiveAttention (QKV projection + causal
sliding-window attention, span=128) on 8 NeuronCores.

Sharding: sequence-parallel with a 1-block halo -- 8 shards of
(batch b, sequence half h): each core owns 2048 query tokens and receives
128 halo tokens of x so attention needs no collectives (the window is
local).

Algorithmic restructure (vs the straightforward q = xWq^T+bq, k = xWk^T+bk,
s = q k^T): the score matrix only ever needs K through q k^T, so the host
precomputes M^T = Wq^T Wk and bt = Wk^T bq, and the kernel computes

    z[d1, q] = sum_d2 M^T[d2, d1] x_q[d2, q] + bt[d1]      (= "Q" in W-space)
    s[k, q]  = sum_d1 x_kv[d1, k] z[d1, q]                 (scores)

which equals q k^T + (terms constant in k). Constant-in-k terms cancel in
softmax, and the k-dependent bias cross-term folds exactly into bt. This
eliminates the entire K projection (a third of the projection FLOPs);
x_kv is just the raw input tile that is already in SBUF (the halo columns
come from the previous chunk's x tile, which ping-pongs).

Per-core layout strategy:
  - all matmuls in f16 operands (2^-11 rounding, below the accumulated
    matmul rounding); f32 PSUM accumulation.
  - x is passed pre-transposed AND pre-tiled per chunk ([128, d-major
    chunk blocks]) so every x DMA moves 128 lines of 4-6KB (the DGE is
    packet-rate-bound: 512B lines halve its effective bandwidth).
  - z is computed transposed (z^T in PSUM), drained with the bt bias add;
    scores sT[k, q] use x tiles as lhsT and z as rhs.
  - packed 512-col score layout (kv-tile r0 only vs the first 128-query
    half, r1 vs both, r2 only vs the second half), one PSUM bank per
    query half so each half's mask+exp only waits on its own chains;
    softmax denominators from a ones-matmul into the same PSUM tile.
  - mask+exp run per query half (DVE scale+mask, ACT exp); the output
    normalize is an ACT Copy-with-per-partition-scale straight out of
    PSUM, split in two so the two output-DMA halves go to the two
    hardware DGE queues (Sync + Scalar) in parallel.
  - input DMAs alternate between the two DGE queues (each descriptor
    submit costs ~620ns of engine time, and one queue sustains only
    ~280 GB/s); chunk-0's z/V projections run d-OUTER with all output
    tiles resident in PSUM so the PE starts on the first (x, M) d-tiles.
  - output is stored f16 and converted on the host.
"""

import sys

if "/opt/trn_rl_repo" not in sys.path:
    sys.path.insert(0, "/opt/trn_rl_repo")

import numpy as np
from contextlib import ExitStack

import os

import concourse.bass as bass
import concourse.mybir as mybir
import concourse.tile as tile
from concourse.bass_utils import run_bass_kernel_spmd
from concourse.vector_clock import ScopedClock

_ORIG_RUN = run_bass_kernel_spmd

# ---------------------------------------------------------------------------
# Problem constants (hardcoded per spec)
B, T, D = 4, 4096, 1024
SPAN = 128
NCORES = 8
TOKQ = T // 2           # 2048 query tokens per core
HALO = SPAN             # 128
CH = 256                # query chunk size (2 blocks = 1 block-pair)
NCH = TOKQ // CH        # 8 chunks
KVW = CH + HALO         # 384 KV tokens visible per chunk
DT = D // 128           # 8 d-tiles
NEG = -1.0e9
SCALE = 1.0 / 32.0      # 1/sqrt(D)

F32 = mybir.dt.float32
F32R = mybir.dt.float32r
F16 = mybir.dt.float16

# ---------------------------------------------------------------------------
# Walrus in this toolchain caps semaphore waits per instruction; Tile's
# kernel-tail Drain can exceed it. Chunk excess waits onto extra drains.
_MAX_WAITS = 1


def _patched_drain_and_barrier(self, tick_clock, wait_clock):
    nc = self.nc
    drain_inst = nc.sync.drain()
    wait_clock.add_sem_waits(
        drain_inst.ins, ScopedClock({None: tick_clock.global_clock})
    )
    si = drain_inst.ins.sync_info
    if si is not None and len(si.on_wait) > _MAX_WAITS:
        waits = list(si.on_wait)
        si.on_wait[:] = waits[:_MAX_WAITS]
        rest = waits[_MAX_WAITS:]
        while rest:
            extra = nc.sync.drain(fusable=False)
            extra.ins.sync_info = mybir.SyncInfo(
                on_wait=rest[:_MAX_WAITS], on_update=[]
            )
            rest = rest[_MAX_WAITS:]
    nc.all_engine_barrier()
    assert self.sems is not None
    popped = nc._tile_sem_poison_stack.pop()
    assert popped is self._sem_poison
    nc.clear_and_free_semaphores(list(self.sems.allocated().values()))
    nc.all_engine_barrier()


def _install_drain_patch():
    if getattr(tile.TileContext, "_drain_patch_installed", False):
        return
    tile.TileContext._drain_and_barrier = _patched_drain_and_barrier
    tile.TileContext._drain_patch_installed = True


def _split_multi_waits(nc, max_waits=_MAX_WAITS):
    """Walrus here supports one semaphore wait per instruction; hoist excess
    waits onto same-engine NoOps inserted immediately before."""
    for fn in nc.m.functions:
        for bb in fn.blocks:
            insts = bb.instructions
            out = []
            changed = False
            for inst in insts:
                si = getattr(inst, "sync_info", None)
                waits = list(si.on_wait) if si is not None else []
                if len(waits) > max_waits:
                    changed = True
                    for w in waits[:-max_waits]:
                        out.append(mybir.InstNoOp(
                            name=nc.get_next_instruction_name(),
                            sync_info=mybir.SyncInfo(on_wait=[w], on_update=[]),
                            bass_nofuse=True,
                            engine=inst.engine,
                        ))
                    si.on_wait[:] = waits[-max_waits:]
                out.append(inst)
            if changed:
                bb.instructions = out


# ---------------------------------------------------------------------------
def _build_graph():
    """Build the per-core Bass graph (SPMD: identical on all 8 cores)."""
    _install_drain_patch()
    nc = bass.Bass()

    # x, chunk-blocked: block 0 = [8 d-tiles x 384 cols] (halo+own of chunk
    # 0), blocks 1..7 = [8 x 256] (own tokens of chunks 1..7), concatenated
    # along the free dim; each 128-partition row is contiguous per block.
    XB0 = DT * KVW                  # 3072 cols in block 0
    XBC = DT * CH                   # 2048 cols per later block
    xch = nc.declare_dram_parameter(
        "xch", [128, XB0 + (NCH - 1) * XBC], F16, isOutput=False)
    mT = nc.declare_dram_parameter("mT", [D, D], F16, isOutput=False)
    wvT = nc.declare_dram_parameter("wvT", [D, D], F16, isOutput=False)
    bt8 = nc.declare_dram_parameter("bt8", [128, DT], F32, isOutput=False)
    # packed: constf = [bv | maskc | maskf]
    constf = nc.declare_dram_parameter("constf", [128, 2 * D], F32, isOutput=False)
    onesp = nc.declare_dram_parameter("onesp", [128, 8], F16, isOutput=False)
    out = nc.declare_dram_parameter("out", [TOKQ, D], F16, isOutput=True)

    with ExitStack() as ctx:
        tc = ctx.enter_context(tile.TileContext(nc))
        consts = ctx.enter_context(tc.tile_pool(name="consts", bufs=1))
        persist = ctx.enter_context(tc.tile_pool(name="persist", bufs=1))
        xc0p = ctx.enter_context(tc.tile_pool(name="xc0p", bufs=1))
        # bufs=3: chunk c's x tile stays alive through chunk c+1's scores
        # (halo); with only 2 bufs, chunk c+2's x DMA would wait for that
        # release and land ~3us after its z projection wants it.
        xcp = ctx.enter_context(tc.tile_pool(name="xcp", bufs=3))
        work = ctx.enter_context(tc.tile_pool(name="work", bufs=2))
        outp = ctx.enter_context(tc.tile_pool(name="outp", bufs=2))
        psp = ctx.enter_context(tc.tile_pool(name="psp", bufs=2, space="PSUM"))
        pss = ctx.enter_context(tc.tile_pool(name="pss", bufs=1, space="PSUM"))
        pso = ctx.enter_context(tc.tile_pool(name="pso", bufs=2, space="PSUM"))

        # --- warmup + resident weights/constants ------------------------
        warm = consts.tile([128, 128], F16, tag="warm")
        nc.vector.memset(warm, 0.0)
        # Score PSUM: one separate tile (= one bank) per 128-query half so
        # each half's mask+exp only waits on its own chains (Tile tracks
        # dependencies per tile); cols 256:264 hold the half's softmax
        # denominator. The warmup matmuls borrow ps_sA (free until the
        # chunk-0 scores).
        ps_sA = pss.tile([128, 512], F32, tag="sTA")
        ps_sB = pss.tile([128, 512], F32, tag="sTB")
        for _ in range(26):
            nc.tensor.matmul(ps_sA[:, 0:128], warm, warm, start=True, stop=True)

        # chunk-0 x in three submits (d0 alone so the first z matmuls start
        # earliest), M / Wv d-tiles alternating across the two DGE queues
        # in need-order; chunk-1's x on the lighter Scalar queue so its
        # hoisted z projection (below) can run before chunk-0's V.
        xc0 = xc0p.tile([128, DT, KVW], F16, tag="xc0")
        nc.sync.dma_start(out=xc0[:, 0, :], in_=xch[:, 0:KVW])
        mT_sb, wv_sb = [], []
        for d in range(DT):
            m = consts.tile([128, D], F16, tag=f"mT{d}")
            (nc.sync if d % 2 else nc.scalar).dma_start(
                out=m, in_=mT[d * 128:(d + 1) * 128, :])
            mT_sb.append(m)
            if d == 1:
                nc.sync.dma_start(
                    out=xc0[:, 1:4, :],
                    in_=xch[:, KVW:4 * KVW].rearrange("p (d c) -> p d c", d=3))
            if d == 2:
                nc.scalar.dma_start(
                    out=xc0[:, 4:8, :],
                    in_=xch[:, 4 * KVW:XB0].rearrange("p (d c) -> p d c", d=4))
        bt_sb = consts.tile([128, DT], F32, tag="bt8")
        nc.scalar.dma_start(out=bt_sb, in_=bt8[:, :])
        xc1 = xcp.tile([128, DT, CH], F16, tag="xc")
        nc.scalar.dma_start(
            out=xc1[:, :, :],
            in_=xch[:, XB0:XB0 + XBC].rearrange("p (d c) -> p d c", d=DT))
        for d in range(DT):
            wv = consts.tile([128, D], F16, tag=f"wv{d}")
            (nc.sync if d % 2 == 0 else nc.scalar).dma_start(
                out=wv, in_=wvT[d * 128:(d + 1) * 128, :])
            wv_sb.append(wv)
        cf_sb = consts.tile([128, 2 * D], F32, tag="constf")
        nc.scalar.dma_start(out=cf_sb, in_=constf[:, :])
        bv_sb = cf_sb[:, 0:D]
        mask_sb = cf_sb[:, D:D + 512]
        maskf_sb = cf_sb[:, D + 512:D + 1024]
        ones_sb = consts.tile([128, 8], F16, tag="ones")
        nc.scalar.dma_start(out=ones_sb, in_=onesp[:, :])

        # --- persistent per-chunk state (ping-pong) ---------------------
        # zT: [128, d1_tile, q], V: [128, tok_tile(3), e]
        zT2 = [persist.tile([128, DT, CH], F16, tag=f"zT{i}", name=f"zT{i}")
               for i in range(2)]
        V2 = [persist.tile([128, KVW // 128, D], F16, tag=f"V{i}", name=f"V{i}")
              for i in range(2)]

        xc_prev = None
        for c in range(NCH):
            zT, V = zT2[c % 2], V2[c % 2]
            Vp = V2[1 - c % 2]
            # ---- x chunk DMA (one submit, 4KB lines) -------------------
            if c == 0:
                xc = xc0
                own0 = HALO
                kv_t0 = 0
            elif c == 1:
                xc = xc1
                own0 = 0
                kv_t0 = 1
                nc.gpsimd.tensor_copy(V[:, 0, :], Vp[:, 2, :])
            else:
                xc = xcp.tile([128, DT, CH], F16, tag="xc")
                lo = XB0 + (c - 1) * XBC
                nc.sync.dma_start(
                    out=xc[:, :, :],
                    in_=xch[:, lo:lo + XBC].rearrange("p (d c) -> p d c", d=DT))
                own0 = 0
                kv_t0 = 1
                # previous chunk's last 128 V rows become this chunk's halo
                nc.gpsimd.tensor_copy(V[:, 0, :], Vp[:, 2, :])

            # k-tile lhsT sources for the score matmuls: (tile, col0) for
            # kv tiles 0 (halo), 1, 2 -- the halo columns live in the
            # previous chunk's x tile (ping-pong keeps it alive).
            if c == 0:
                ksrc = [(xc0, 0), (xc0, 128), (xc0, 256)]
            else:
                hal0 = KVW - HALO if c == 1 else CH - HALO
                ksrc = [(xc_prev, hal0), (xc, 0), (xc, 128)]

            # ---- z projection: zT = M^T-chain over d2, + bt bias -------
            def z_proj_eouter(xq, zT_):
                for e in range(DT):
                    ps = psp.tile([128, 512], F32, tag="proj")
                    for d in range(DT):
                        nc.tensor.matmul(
                            ps[:, 0:CH],
                            mT_sb[d][:, e * 128:(e + 1) * 128],
                            xq[:, d, 0:CH],
                            start=(d == 0),
                            stop=(d == DT - 1),
                        )
                    nc.vector.tensor_scalar_add(
                        zT_[:, e, :], ps[:, 0:CH], bt_sb[:, e:e + 1])

            if c == 0:
                # d-OUTER for e0-e5: accumulators resident in PSUM so the
                # first matmuls need only the first (x, M) d-tiles. ONE
                # accumulator per PSUM bank: a matmul with start=True
                # clears the whole destination bank's has_written bits, so
                # two interleaved accumulation chains must never share a
                # bank. e6/e7 run as ordinary chains after (all of M has
                # landed by then), leaving the score banks free for the
                # dummy matmuls that absorb DMA jitter between d-steps
                # (PE idle >3.4us here would re-throttle the HAM clock).
                psA = psp.tile([128, 512], F32, tag="proj")
                psB = psp.tile([128, 512], F32, tag="proj")
                psC = pso.tile([128, 1024], F32, tag="o")
                psD = pso.tile([128, 1024], F32, tag="o")
                zreg = [
                    (0, psA[:, 0:256]), (1, psB[:, 0:256]),
                    (2, psC[:, 0:256]), (3, psC[:, 512:768]),
                    (4, psD[:, 0:256]), (5, psD[:, 512:768]),
                ]
                for d in range(DT):
                    for e, reg in zreg:
                        nc.tensor.matmul(
                            reg,
                            mT_sb[d][:, e * 128:(e + 1) * 128],
                            xc[:, d, own0:own0 + CH],
                            start=(d == 0),
                            stop=(d == DT - 1),
                        )
                    if d < 6:
                        for _ in range(4):
                            nc.tensor.matmul(
                                ps_sA[:, 0:128], warm, warm,
                                start=True, stop=True)
                for e, reg in zreg:
                    nc.vector.tensor_scalar_add(
                        zT[:, e, :], reg, bt_sb[:, e:e + 1])
                for e, reg in ((6, ps_sA[:, 0:256]), (7, ps_sB[:, 0:256])):
                    for d in range(DT):
                        nc.tensor.matmul(
                            reg,
                            mT_sb[d][:, e * 128:(e + 1) * 128],
                            xc[:, d, own0:own0 + CH],
                            start=(d == 0),
                            stop=(d == DT - 1),
                        )
                    nc.vector.tensor_scalar_add(
                        zT[:, e, :], reg, bt_sb[:, e:e + 1])
                # chunk-1's z, software-pipelined here: it only needs the
                # resident M tiles + xc1 (landed long ago on the Scalar
                # queue), so the PE stays busy while Wv streams in for the
                # V projection below (no HAM re-throttle).
                z_proj_eouter(xc1, zT2[1])
            elif c > 1:
                z_proj_eouter(xc, zT)

            # ---- V projection (last psp user of the chunk: its drains
            # are long done when the next chunk's z wants the slots) -----
            if c == 0:
                # d-OUTER: 6 accumulators in pso+psp
                vs0 = pso.tile([128, 1024], F32, tag="o")
                vs1 = pso.tile([128, 1024], F32, tag="o")
                vs2 = psp.tile([128, 512], F32, tag="proj")
                vs3 = psp.tile([128, 512], F32, tag="proj")
                vreg = [
                    (0, 0, vs0[:, 0:512]), (0, 1, vs0[:, 512:1024]),
                    (1, 0, vs1[:, 0:512]), (1, 1, vs1[:, 512:1024]),
                    (2, 0, vs2), (2, 1, vs3),
                ]
                for d in range(DT):
                    for t, eh, reg in vreg:
                        nc.tensor.matmul(
                            reg,
                            xc[:, d, t * 128:(t + 1) * 128],
                            wv_sb[d][:, eh * 512:(eh + 1) * 512],
                            start=(d == 0),
                            stop=(d == DT - 1),
                        )
                for t, eh, reg in vreg:
                    nc.vector.tensor_add(
                        V[:, t, eh * 512:(eh + 1) * 512],
                        reg,
                        bv_sb[:, eh * 512:(eh + 1) * 512],
                    )
            else:
                for t in range(1, KVW // 128):
                    xcol = (t - 1) * 128
                    for eh in range(2):
                        ps = psp.tile([128, 512], F32, tag="proj")
                        for d in range(DT):
                            nc.tensor.matmul(
                                ps,
                                xc[:, d, xcol:xcol + 128],
                                wv_sb[d][:, eh * 512:(eh + 1) * 512],
                                start=(d == 0),
                                stop=(d == DT - 1),
                            )
                        nc.vector.tensor_add(
                            V[:, t, eh * 512:(eh + 1) * 512],
                            ps,
                            bv_sb[:, eh * 512:(eh + 1) * 512],
                        )

            # ---- attention -------------------------------------------
            # Packed score layout, one PSUM TILE (= one bank) per query
            # half: cols 0:128 prev-kv-tile scores, 128:256 own-kv-tile
            # scores, 256:264 the half's softmax denominator.
            ps_sA = pss.tile([128, 512], F32, tag="sTA")
            ps_sB = pss.tile([128, 512], F32, tag="sTB")
            schains = [
                (ps_sA, slice(0, 128), 0, slice(0, 128)),      # r0 qA
                (ps_sA, slice(128, 256), 1, slice(0, 128)),    # r1 qA
                (ps_sB, slice(0, 128), 1, slice(128, 256)),    # r1 qB
                (ps_sB, slice(128, 256), 2, slice(128, 256)),  # r2 qB
            ]
            for ps_sX, scol, kt, qsl in schains:
                lsrc, kc = ksrc[kt]
                for d1 in range(DT):
                    nc.tensor.matmul(
                        ps_sX[:, scol],
                        lsrc[:, d1, kc:kc + 128], zT[:, d1, qsl],
                        start=(d1 == 0), stop=(d1 == DT - 1),
                    )
            # wT = exp(sT * scale + mask), per query half (DVE then ACT)
            m_sb = maskf_sb if c == 0 else mask_sb
            stt = work.tile([128, 2 * CH], F32, tag="stt")
            wT = work.tile([128, 2 * CH], F16, tag="wT")
            for half, ps_sX in ((0, ps_sA), (1, ps_sB)):
                hsl = slice(half * 256, half * 256 + 256)
                nc.vector.scalar_tensor_tensor(
                    stt[:, hsl], ps_sX[:, 0:256], SCALE, m_sb[:, hsl],
                    mybir.AluOpType.mult, mybir.AluOpType.add,
                )
                nc.scalar.activation(
                    wT[:, hsl], stt[:, hsl], mybir.ActivationFunctionType.Exp
                )

            # Denominator matmuls FIRST so the reciprocal (DVE) overlaps
            # the o matmuls; normalize = Copy-with-scale straight from
            # PSUM (one e-half on ACT, one on DVE), out-DMA split across
            # the two DGE queues.
            for half, ps_sX in ((0, ps_sA), (1, ps_sB)):
                ps_o = pso.tile([128, 1024], F32, tag="o")
                for ri in range(2):
                    lhsT = wT[:, half * 256 + ri * 128:
                              half * 256 + ri * 128 + 128]
                    nc.tensor.matmul(
                        ps_sX[:, 256:264],
                        lhsT,
                        ones_sb,
                        start=(ri == 0),
                        stop=(ri == 1),
                    )
                recip = outp.tile([128, 1], F32, tag="recip")
                nc.vector.reciprocal(recip, ps_sX[:, 256:257])
                for ri in range(2):
                    lhsT = wT[:, half * 256 + ri * 128:
                              half * 256 + ri * 128 + 128]
                    r = half + ri     # V token-tile index
                    for eh in range(2):
                        nc.tensor.matmul(
                            ps_o[:, eh * 512:(eh + 1) * 512],
                            lhsT,
                            V[:, r, eh * 512:(eh + 1) * 512],
                            start=(ri == 0),
                            stop=(ri == 1),
                        )
                o_sb = outp.tile([128, D], F16, tag="o_sb")
                row0 = c * CH + half * 128
                # normalize+store in [128,256] quarters, alternating
                # ACT/DVE and the two DGE queues, so the last chunk's
                # epilogue chain is a quarter as deep
                for qt in range(4):
                    qsl2 = slice(qt * 256, qt * 256 + 256)
                    if qt % 2 == 0:
                        nc.scalar.activation(
                            o_sb[:, qsl2], ps_o[:, qsl2],
                            mybir.ActivationFunctionType.Copy, scale=recip,
                        )
                        nc.sync.dma_start(
                            out=out[row0:row0 + 128, qsl2], in_=o_sb[:, qsl2]
                        )
                    else:
                        nc.vector.tensor_scalar_mul(
                            o_sb[:, qsl2], ps_o[:, qsl2], recip
                        )
                        nc.scalar.dma_start(
                            out=out[row0:row0 + 128, qsl2], in_=o_sb[:, qsl2]
                        )
            xc_prev = xc

    _split_multi_waits(nc)
    return nc


_GRAPH = None


def _get_graph():
    global _GRAPH
    if _GRAPH is None:
        _GRAPH = _build_graph()
    return _GRAPH


# Cached jitted executable: run_bass_via_pjrt rebuilds its jit closure per
# call (a full retrace each time); replicate its multi-core path once and
# reuse it so repeated kernel() calls cost only the device execution.
_RUNNER = None


def _fast_run(nc, in_maps):
    global _RUNNER
    import jax
    from jax.experimental.shard_map import shard_map
    from jax.sharding import Mesh, PartitionSpec
    from concourse import bass2jax

    n_cores = len(in_maps)
    if _RUNNER is None:
        bass2jax.install_neuronx_cc_hook()
        partition_name = (
            nc.partition_id_tensor.name if nc.partition_id_tensor else None
        )
        in_names, out_names, out_avals = [], [], []
        for alloc in nc.m.functions[0].allocations:
            if not isinstance(alloc, mybir.MemoryLocationSet):
                continue
            name = alloc.memorylocations[0].name
            if alloc.kind == "ExternalInput":
                if name != partition_name:
                    in_names.append(name)
            elif alloc.kind == "ExternalOutput":
                out_names.append(name)
                out_avals.append(jax.core.ShapedArray(
                    tuple(alloc.tensor_shape), mybir.dt.np(alloc.dtype)))
        n_params = len(in_names)
        all_names = in_names + out_names
        if partition_name is not None:
            all_names.append(partition_name)
        donate = tuple(range(n_params, n_params + len(out_names)))

        def _body(*args):
            operands = list(args)
            if partition_name is not None:
                operands.append(bass2jax.partition_id_tensor())
            return tuple(bass2jax._bass_exec_p.bind(
                *operands,
                out_avals=tuple(out_avals),
                in_names=tuple(all_names),
                out_names=tuple(out_names),
                lowering_input_output_aliases=(),
                sim_require_finite=True,
                sim_require_nnan=True,
                nc=nc,
            ))

        devices = jax.devices()[:n_cores]
        mesh = Mesh(np.asarray(devices), ("core",))
        nio = n_params + len(out_names)
        sharded = jax.jit(
            shard_map(
                _body, mesh=mesh,
                in_specs=(PartitionSpec("core"),) * nio,
                out_specs=(PartitionSpec("core"),) * len(out_names),
                check_rep=False,
            ),
            donate_argnums=donate,
            keep_unused=True,
        )
        # allocate the donated output buffers on-device (no host transfer)
        import jax.numpy as jnp
        from jax.sharding import NamedSharding
        shardings = tuple(
            NamedSharding(mesh, PartitionSpec("core")) for _ in out_avals
        )
        zeros_jit = jax.jit(
            lambda: tuple(
                jnp.zeros((n_cores * av.shape[0], *av.shape[1:]), av.dtype)
                for av in out_avals
            ),
            out_shardings=shardings,
        )
        _RUNNER = (sharded, zeros_jit, in_names, out_names, out_avals)

    sharded, zeros_jit, in_names, out_names, out_avals = _RUNNER
    concat_in = [
        np.concatenate([np.asarray(m[name]) for m in in_maps], axis=0)
        for name in in_names
    ]
    out_arrs = sharded(*concat_in, *zeros_jit())
    return [
        {
            name: np.asarray(out_arrs[i]).reshape(
                n_cores, *out_avals[i].shape)[c]
            for i, name in enumerate(out_names)
        }
        for c in range(n_cores)
    ]


# ---------------------------------------------------------------------------
def _make_masks():
    """Additive masks in the packed 512-col layout (post-scale):
    [r0 vs qA | r1 vs qA | r1 vs qB | r2 vs qB], 128 q-cols each."""
    kp = np.arange(128)[:, None]
    qf = np.arange(128)[None, :]
    upper = np.where(kp > qf, 0.0, NEG)    # strict upper triangle kept
    lower = np.where(kp <= qf, 0.0, NEG)   # lower (incl diag) kept
    m = np.concatenate([upper, lower, upper, lower], axis=1).astype(np.float32)
    mf = m.copy()
    mf[:, 0:128] = NEG  # global block 0: the halo "previous block" is padding
    return m, mf


def kernel(x, Wq, bq, Wk, bk, Wv, bv, span):
    x = np.asarray(x)
    span_i = int(np.asarray(span))
    assert span_i == SPAN, f"kernel hardcodes span={SPAN}, got {span_i}"
    assert x.shape == (B, T, D)

    nc = _get_graph()

    Wq = np.asarray(Wq, dtype=np.float32)
    Wk = np.asarray(Wk, dtype=np.float32)
    # score matrix in x-space: s = x_kv M x_q^T with M = Wk^T Wq; the
    # kernel's z-chain lhsT wants mT[d2, d1] = M[d1, d2] = (Wq^T Wk)[d2, d1]
    mTh = np.ascontiguousarray(Wq.T @ Wk).astype(np.float16)
    bt = Wk.T @ np.asarray(bq, dtype=np.float32)          # [D], per-d1 bias
    bt8 = np.ascontiguousarray(bt.reshape(DT, 128).T).astype(
        np.float32, copy=False)
    wvT = np.ascontiguousarray(np.asarray(Wv).T).astype(np.float16)
    bvb = np.broadcast_to(np.asarray(bv), (128, D))
    m, mf = _make_masks()
    # packed [bv | maskc | maskf] -- one DMA instead of three
    cfc = np.ascontiguousarray(
        np.concatenate([bvb, m, m], axis=1)).astype(np.float32, copy=False)
    cff = np.ascontiguousarray(
        np.concatenate([bvb, m, mf], axis=1)).astype(np.float32, copy=False)
    ones = np.ones((128, 8), np.float16)

    in_maps = []
    for core in range(NCORES):
        b, h = divmod(core, 2)
        lo = h * TOKQ - HALO
        hi = (h + 1) * TOKQ
        xs = np.zeros((HALO + TOKQ, D), np.float32)
        if lo < 0:
            xs[HALO:] = x[b, 0:hi]
        else:
            xs[:] = x[b, lo:hi]
        # chunk-blocked x: [128, d, tok] per chunk block, d-major columns
        # (4-6KB contiguous per partition row per block -> fat DMA lines)
        xt3 = np.ascontiguousarray(
            xs.T.astype(np.float16).reshape(DT, 128, HALO + TOKQ)
            .transpose(1, 0, 2))          # [128, DT, HALO+TOKQ]
        blocks = [xt3[:, :, 0:KVW].reshape(128, DT * KVW)]
        for c in range(1, NCH):
            lo_c = HALO + c * CH
            blocks.append(
                xt3[:, :, lo_c:lo_c + CH].reshape(128, DT * CH))
        xch = np.ascontiguousarray(np.concatenate(blocks, axis=1))
        in_maps.append({
            "xch": xch, "mT": mTh, "wvT": wvT, "bt8": bt8,
            "constf": (cff if h == 0 else cfc), "onesp": ones,
        })

    if run_bass_kernel_spmd is _ORIG_RUN and not os.environ.get("BASS_TRACE"):
        results = _fast_run(nc, in_maps)
    else:
        # a harness monkeypatched run_bass_kernel_spmd (e.g. for tracing)
        results = run_bass_kernel_spmd(
            nc, in_maps, core_ids=list(range(NCORES))
        ).results

    out = np.empty((B, T, D), np.float32)
    for core in range(NCORES):
        b, h = divmod(core, 2)
        out[b, h * TOKQ:(h + 1) * TOKQ] = results[core]["out"].astype(
            np.float32
        )
    return out



# revision 21
# speedup vs baseline: 1.0223x; 1.0223x over previous
"""Trainium2 kernel for AdaptiveAttention (QKV projection + causal
sliding-window attention, span=128) on 8 NeuronCores.

Sharding: sequence-parallel with a 1-block halo -- 8 shards of
(batch b, sequence half h): each core owns 2048 query tokens and receives
128 halo tokens of x so attention needs no collectives (the window is
local).

Algorithmic restructure (vs the straightforward q = xWq^T+bq, k = xWk^T+bk,
s = q k^T): the score matrix only ever needs K through q k^T, so the host
precomputes M^T = Wq^T Wk and bt = Wk^T bq, and the kernel computes

    z[d1, q] = sum_d2 M^T[d2, d1] x_q[d2, q] + bt[d1]      (= "Q" in W-space)
    s[k, q]  = sum_d1 x_kv[d1, k] z[d1, q]                 (scores)

which equals q k^T + (terms constant in k). Constant-in-k terms cancel in
softmax, and the k-dependent bias cross-term folds exactly into bt. This
eliminates the entire K projection (a third of the projection FLOPs);
x_kv is just the raw input tile that is already in SBUF (the halo columns
come from the previous chunk's x tile, which ping-pongs).

Per-core layout strategy:
  - all matmuls in f16 operands (2^-11 rounding, below the accumulated
    matmul rounding); f32 PSUM accumulation.
  - x is passed pre-transposed AND pre-tiled per chunk ([128, d-major
    chunk blocks]) so every x DMA moves 128 lines of 4-6KB (the DGE is
    packet-rate-bound: 512B lines halve its effective bandwidth).
  - z is computed transposed (z^T in PSUM), drained with the bt bias add;
    scores sT[k, q] use x tiles as lhsT and z as rhs.
  - packed 512-col score layout (kv-tile r0 only vs the first 128-query
    half, r1 vs both, r2 only vs the second half), one PSUM bank per
    query half so each half's mask+exp only waits on its own chains;
    softmax denominators from a ones-matmul into the same PSUM tile.
  - mask+exp run per query half (DVE scale+mask, ACT exp); the output
    normalize is an ACT Copy-with-per-partition-scale straight out of
    PSUM, split in two so the two output-DMA halves go to the two
    hardware DGE queues (Sync + Scalar) in parallel.
  - input DMAs alternate between the two DGE queues (each descriptor
    submit costs ~620ns of engine time, and one queue sustains only
    ~280 GB/s); chunk-0's z/V projections run d-OUTER with all output
    tiles resident in PSUM so the PE starts on the first (x, M) d-tiles.
  - output is stored f16 and converted on the host.
"""

import sys

if "/opt/trn_rl_repo" not in sys.path:
    sys.path.insert(0, "/opt/trn_rl_repo")

import numpy as np
from contextlib import ExitStack

import os

import concourse.bass as bass
import concourse.mybir as mybir
import concourse.tile as tile
from concourse.bass_utils import run_bass_kernel_spmd
from concourse.vector_clock import ScopedClock

_ORIG_RUN = run_bass_kernel_spmd

# ---------------------------------------------------------------------------
# Problem constants (hardcoded per spec)
B, T, D = 4, 4096, 1024
SPAN = 128
NCORES = 8
TOKQ = T // 2           # 2048 query tokens per core
HALO = SPAN             # 128
CH = 256                # query chunk size (2 blocks = 1 block-pair)
NCH = TOKQ // CH        # 8 chunks
KVW = CH + HALO         # 384 KV tokens visible per chunk
DT = D // 128           # 8 d-tiles
NEG = -1.0e9
SCALE = 1.0 / 32.0      # 1/sqrt(D)

F32 = mybir.dt.float32
F32R = mybir.dt.float32r
F16 = mybir.dt.float16

# ---------------------------------------------------------------------------
# Walrus in this toolchain caps semaphore waits per instruction; Tile's
# kernel-tail Drain can exceed it. Chunk excess waits onto extra drains.
_MAX_WAITS = 1


def _patched_drain_and_barrier(self, tick_clock, wait_clock):
    nc = self.nc
    drain_inst = nc.sync.drain()
    wait_clock.add_sem_waits(
        drain_inst.ins, ScopedClock({None: tick_clock.global_clock})
    )
    si = drain_inst.ins.sync_info
    if si is not None and len(si.on_wait) > _MAX_WAITS:
        waits = list(si.on_wait)
        si.on_wait[:] = waits[:_MAX_WAITS]
        rest = waits[_MAX_WAITS:]
        while rest:
            extra = nc.sync.drain(fusable=False)
            extra.ins.sync_info = mybir.SyncInfo(
                on_wait=rest[:_MAX_WAITS], on_update=[]
            )
            rest = rest[_MAX_WAITS:]
    nc.all_engine_barrier()
    assert self.sems is not None
    popped = nc._tile_sem_poison_stack.pop()
    assert popped is self._sem_poison
    nc.clear_and_free_semaphores(list(self.sems.allocated().values()))
    nc.all_engine_barrier()


def _install_drain_patch():
    if getattr(tile.TileContext, "_drain_patch_installed", False):
        return
    tile.TileContext._drain_and_barrier = _patched_drain_and_barrier
    tile.TileContext._drain_patch_installed = True


def _split_multi_waits(nc, max_waits=_MAX_WAITS):
    """Walrus here supports one semaphore wait per instruction; hoist excess
    waits onto same-engine NoOps inserted immediately before."""
    for fn in nc.m.functions:
        for bb in fn.blocks:
            insts = bb.instructions
            out = []
            changed = False
            for inst in insts:
                si = getattr(inst, "sync_info", None)
                waits = list(si.on_wait) if si is not None else []
                if len(waits) > max_waits:
                    changed = True
                    for w in waits[:-max_waits]:
                        out.append(mybir.InstNoOp(
                            name=nc.get_next_instruction_name(),
                            sync_info=mybir.SyncInfo(on_wait=[w], on_update=[]),
                            bass_nofuse=True,
                            engine=inst.engine,
                        ))
                    si.on_wait[:] = waits[-max_waits:]
                out.append(inst)
            if changed:
                bb.instructions = out


# ---------------------------------------------------------------------------
def _build_graph():
    """Build the per-core Bass graph (SPMD: identical on all 8 cores)."""
    _install_drain_patch()
    nc = bass.Bass()

    # x, chunk-blocked: block 0 = [8 d-tiles x 384 cols] (halo+own of chunk
    # 0), blocks 1..7 = [8 x 256] (own tokens of chunks 1..7), concatenated
    # along the free dim; each 128-partition row is contiguous per block.
    XB0 = DT * KVW                  # 3072 cols in block 0
    XBC = DT * CH                   # 2048 cols per later block
    xch = nc.declare_dram_parameter(
        "xch", [128, XB0 + (NCH - 1) * XBC], F16, isOutput=False)
    mT = nc.declare_dram_parameter("mT", [D, D], F16, isOutput=False)
    wvT = nc.declare_dram_parameter("wvT", [D, D], F16, isOutput=False)
    bt8 = nc.declare_dram_parameter("bt8", [128, DT], F32, isOutput=False)
    # packed: constf = [bv | maskc | maskf]
    constf = nc.declare_dram_parameter("constf", [128, 2 * D], F32, isOutput=False)
    onesp = nc.declare_dram_parameter("onesp", [128, 8], F16, isOutput=False)
    out = nc.declare_dram_parameter("out", [TOKQ, D], F16, isOutput=True)

    with ExitStack() as ctx:
        tc = ctx.enter_context(tile.TileContext(nc))
        consts = ctx.enter_context(tc.tile_pool(name="consts", bufs=1))
        persist = ctx.enter_context(tc.tile_pool(name="persist", bufs=1))
        xc0p = ctx.enter_context(tc.tile_pool(name="xc0p", bufs=1))
        # bufs=3: chunk c's x tile stays alive through chunk c+1's scores
        # (halo); with only 2 bufs, chunk c+2's x DMA would wait for that
        # release and land ~3us after its z projection wants it.
        xcp = ctx.enter_context(tc.tile_pool(name="xcp", bufs=3))
        work = ctx.enter_context(tc.tile_pool(name="work", bufs=2))
        outp = ctx.enter_context(tc.tile_pool(name="outp", bufs=2))
        psp = ctx.enter_context(tc.tile_pool(name="psp", bufs=2, space="PSUM"))
        pss = ctx.enter_context(tc.tile_pool(name="pss", bufs=1, space="PSUM"))
        pso = ctx.enter_context(tc.tile_pool(name="pso", bufs=2, space="PSUM"))

        # --- warmup + resident weights/constants ------------------------
        warm = consts.tile([128, 128], F16, tag="warm")
        nc.vector.memset(warm, 0.0)
        # Score PSUM: one separate tile (= one bank) per 128-query half so
        # each half's mask+exp only waits on its own chains (Tile tracks
        # dependencies per tile); cols 256:264 hold the half's softmax
        # denominator. The warmup matmuls borrow ps_sA (free until the
        # chunk-0 scores).
        ps_sA = pss.tile([128, 512], F32, tag="sTA")
        ps_sB = pss.tile([128, 512], F32, tag="sTB")
        for _ in range(26):
            nc.tensor.matmul(ps_sA[:, 0:128], warm, warm, start=True, stop=True)

        # chunk-0 x in three submits (d0 alone so the first z matmuls start
        # earliest), M / Wv d-tiles alternating across the two DGE queues
        # in need-order; chunk-1's x on the lighter Scalar queue so its
        # hoisted z projection (below) can run before chunk-0's V.
        xc0 = xc0p.tile([128, DT, KVW], F16, tag="xc0")
        nc.sync.dma_start(out=xc0[:, 0, :], in_=xch[:, 0:KVW])
        mT_sb, wv_sb = [], []
        for d in range(DT):
            m = consts.tile([128, D], F16, tag=f"mT{d}")
            (nc.sync if d % 2 else nc.scalar).dma_start(
                out=m, in_=mT[d * 128:(d + 1) * 128, :])
            mT_sb.append(m)
            if d == 1:
                nc.sync.dma_start(
                    out=xc0[:, 1:4, :],
                    in_=xch[:, KVW:4 * KVW].rearrange("p (d c) -> p d c", d=3))
            if d == 2:
                nc.scalar.dma_start(
                    out=xc0[:, 4:8, :],
                    in_=xch[:, 4 * KVW:XB0].rearrange("p (d c) -> p d c", d=4))
        bt_sb = consts.tile([128, DT], F32, tag="bt8")
        nc.scalar.dma_start(out=bt_sb, in_=bt8[:, :])
        xc1 = xcp.tile([128, DT, CH], F16, tag="xc")
        nc.scalar.dma_start(
            out=xc1[:, :, :],
            in_=xch[:, XB0:XB0 + XBC].rearrange("p (d c) -> p d c", d=DT))
        for d in range(DT):
            wv = consts.tile([128, D], F16, tag=f"wv{d}")
            (nc.sync if d % 2 == 0 else nc.scalar).dma_start(
                out=wv, in_=wvT[d * 128:(d + 1) * 128, :])
            wv_sb.append(wv)
        cf_sb = consts.tile([128, 2 * D], F32, tag="constf")
        nc.scalar.dma_start(out=cf_sb, in_=constf[:, :])
        bv_sb = cf_sb[:, 0:D]
        mask_sb = cf_sb[:, D:D + 512]
        maskf_sb = cf_sb[:, D + 512:D + 1024]
        ones_sb = consts.tile([128, 8], F16, tag="ones")
        nc.scalar.dma_start(out=ones_sb, in_=onesp[:, :])

        # --- persistent per-chunk state (ping-pong) ---------------------
        # zT: [128, d1_tile, q], V: [128, tok_tile(3), e]
        zT2 = [persist.tile([128, DT, CH], F16, tag=f"zT{i}", name=f"zT{i}")
               for i in range(2)]
        V2 = [persist.tile([128, KVW // 128, D], F16, tag=f"V{i}", name=f"V{i}")
              for i in range(2)]

        xc_prev = None
        for c in range(NCH):
            zT, V = zT2[c % 2], V2[c % 2]
            Vp = V2[1 - c % 2]
            # ---- x chunk DMA (one submit, 4KB lines) -------------------
            if c == 0:
                xc = xc0
                own0 = HALO
                kv_t0 = 0
            elif c == 1:
                xc = xc1
                own0 = 0
                kv_t0 = 1
                nc.gpsimd.tensor_copy(V[:, 0, :], Vp[:, 2, :])
            else:
                xc = xcp.tile([128, DT, CH], F16, tag="xc")
                lo = XB0 + (c - 1) * XBC
                nc.sync.dma_start(
                    out=xc[:, :, :],
                    in_=xch[:, lo:lo + XBC].rearrange("p (d c) -> p d c", d=DT))
                own0 = 0
                kv_t0 = 1
                # previous chunk's last 128 V rows become this chunk's halo
                nc.gpsimd.tensor_copy(V[:, 0, :], Vp[:, 2, :])

            # k-tile lhsT sources for the score matmuls: (tile, col0) for
            # kv tiles 0 (halo), 1, 2 -- the halo columns live in the
            # previous chunk's x tile (ping-pong keeps it alive).
            if c == 0:
                ksrc = [(xc0, 0), (xc0, 128), (xc0, 256)]
            else:
                hal0 = KVW - HALO if c == 1 else CH - HALO
                ksrc = [(xc_prev, hal0), (xc, 0), (xc, 128)]

            # ---- z projection: zT = M^T-chain over d2, + bt bias -------
            def z_proj_eouter(xq, zT_):
                for e in range(DT):
                    ps = psp.tile([128, 512], F32, tag="proj")
                    for d in range(DT):
                        nc.tensor.matmul(
                            ps[:, 0:CH],
                            mT_sb[d][:, e * 128:(e + 1) * 128],
                            xq[:, d, 0:CH],
                            start=(d == 0),
                            stop=(d == DT - 1),
                        )
                    nc.vector.tensor_scalar_add(
                        zT_[:, e, :], ps[:, 0:CH], bt_sb[:, e:e + 1])

            if c == 0:
                # d-OUTER for e0-e5: accumulators resident in PSUM so the
                # first matmuls need only the first (x, M) d-tiles. ONE
                # accumulator per PSUM bank: a matmul with start=True
                # clears the whole destination bank's has_written bits, so
                # two interleaved accumulation chains must never share a
                # bank. e6/e7 run as ordinary chains after (all of M has
                # landed by then), leaving the score banks free for the
                # dummy matmuls that absorb DMA jitter between d-steps
                # (PE idle >3.4us here would re-throttle the HAM clock).
                psA = psp.tile([128, 512], F32, tag="proj")
                psB = psp.tile([128, 512], F32, tag="proj")
                psC = pso.tile([128, 1024], F32, tag="o")
                psD = pso.tile([128, 1024], F32, tag="o")
                zreg = [
                    (0, psA[:, 0:256]), (1, psB[:, 0:256]),
                    (2, psC[:, 0:256]), (3, psC[:, 512:768]),
                    (4, psD[:, 0:256]), (5, psD[:, 512:768]),
                ]
                for d in range(DT):
                    for e, reg in zreg:
                        nc.tensor.matmul(
                            reg,
                            mT_sb[d][:, e * 128:(e + 1) * 128],
                            xc[:, d, own0:own0 + CH],
                            start=(d == 0),
                            stop=(d == DT - 1),
                        )
                for e, reg in zreg:
                    nc.vector.tensor_scalar_add(
                        zT[:, e, :], reg, bt_sb[:, e:e + 1])
                for e, reg in ((6, ps_sA[:, 0:256]), (7, ps_sB[:, 0:256])):
                    for d in range(DT):
                        nc.tensor.matmul(
                            reg,
                            mT_sb[d][:, e * 128:(e + 1) * 128],
                            xc[:, d, own0:own0 + CH],
                            start=(d == 0),
                            stop=(d == DT - 1),
                        )
                    nc.vector.tensor_scalar_add(
                        zT[:, e, :], reg, bt_sb[:, e:e + 1])
                # chunk-1's z, software-pipelined here: it only needs the
                # resident M tiles + xc1 (landed long ago on the Scalar
                # queue), so the PE stays busy while Wv streams in for the
                # V projection below (no HAM re-throttle).
                z_proj_eouter(xc1, zT2[1])
            elif c > 1:
                z_proj_eouter(xc, zT)

            # ---- V projection (last psp user of the chunk: its drains
            # are long done when the next chunk's z wants the slots) -----
            if c == 0:
                # d-OUTER: 6 accumulators in pso+psp
                vs0 = pso.tile([128, 1024], F32, tag="o")
                vs1 = pso.tile([128, 1024], F32, tag="o")
                vs2 = psp.tile([128, 512], F32, tag="proj")
                vs3 = psp.tile([128, 512], F32, tag="proj")
                vreg = [
                    (0, 0, vs0[:, 0:512]), (0, 1, vs0[:, 512:1024]),
                    (1, 0, vs1[:, 0:512]), (1, 1, vs1[:, 512:1024]),
                    (2, 0, vs2), (2, 1, vs3),
                ]
                for d in range(DT):
                    for t, eh, reg in vreg:
                        nc.tensor.matmul(
                            reg,
                            xc[:, d, t * 128:(t + 1) * 128],
                            wv_sb[d][:, eh * 512:(eh + 1) * 512],
                            start=(d == 0),
                            stop=(d == DT - 1),
                        )
                for t, eh, reg in vreg:
                    nc.vector.tensor_add(
                        V[:, t, eh * 512:(eh + 1) * 512],
                        reg,
                        bv_sb[:, eh * 512:(eh + 1) * 512],
                    )
            else:
                for t in range(1, KVW // 128):
                    xcol = (t - 1) * 128
                    for eh in range(2):
                        ps = psp.tile([128, 512], F32, tag="proj")
                        for d in range(DT):
                            nc.tensor.matmul(
                                ps,
                                xc[:, d, xcol:xcol + 128],
                                wv_sb[d][:, eh * 512:(eh + 1) * 512],
                                start=(d == 0),
                                stop=(d == DT - 1),
                            )
                        nc.vector.tensor_add(
                            V[:, t, eh * 512:(eh + 1) * 512],
                            ps,
                            bv_sb[:, eh * 512:(eh + 1) * 512],
                        )

            # ---- attention -------------------------------------------
            # Packed score layout, one PSUM TILE (= one bank) per query
            # half: cols 0:128 prev-kv-tile scores, 128:256 own-kv-tile
            # scores, 256:264 the half's softmax denominator.
            ps_sA = pss.tile([128, 512], F32, tag="sTA")
            ps_sB = pss.tile([128, 512], F32, tag="sTB")
            schains = [
                (ps_sA, slice(0, 128), 0, slice(0, 128)),      # r0 qA
                (ps_sA, slice(128, 256), 1, slice(0, 128)),    # r1 qA
                (ps_sB, slice(0, 128), 1, slice(128, 256)),    # r1 qB
                (ps_sB, slice(128, 256), 2, slice(128, 256)),  # r2 qB
            ]
            for ps_sX, scol, kt, qsl in schains:
                lsrc, kc = ksrc[kt]
                for d1 in range(DT):
                    nc.tensor.matmul(
                        ps_sX[:, scol],
                        lsrc[:, d1, kc:kc + 128], zT[:, d1, qsl],
                        start=(d1 == 0), stop=(d1 == DT - 1),
                    )
            # wT = exp(sT * scale + mask), per query half (DVE then ACT)
            m_sb = maskf_sb if c == 0 else mask_sb
            stt = work.tile([128, 2 * CH], F32, tag="stt")
            wT = work.tile([128, 2 * CH], F16, tag="wT")
            for half, ps_sX in ((0, ps_sA), (1, ps_sB)):
                hsl = slice(half * 256, half * 256 + 256)
                nc.vector.scalar_tensor_tensor(
                    stt[:, hsl], ps_sX[:, 0:256], SCALE, m_sb[:, hsl],
                    mybir.AluOpType.mult, mybir.AluOpType.add,
                )
                nc.scalar.activation(
                    wT[:, hsl], stt[:, hsl], mybir.ActivationFunctionType.Exp
                )

            # Denominator matmuls FIRST so the reciprocal (DVE) overlaps
            # the o matmuls; normalize = Copy-with-scale straight from
            # PSUM (one e-half on ACT, one on DVE), out-DMA split across
            # the two DGE queues.
            for half, ps_sX in ((0, ps_sA), (1, ps_sB)):
                ps_o = pso.tile([128, 1024], F32, tag="o")
                for ri in range(2):
                    lhsT = wT[:, half * 256 + ri * 128:
                              half * 256 + ri * 128 + 128]
                    nc.tensor.matmul(
                        ps_sX[:, 256:264],
                        lhsT,
                        ones_sb,
                        start=(ri == 0),
                        stop=(ri == 1),
                    )
                recip = outp.tile([128, 1], F32, tag="recip")
                nc.vector.reciprocal(recip, ps_sX[:, 256:257])
                for ri in range(2):
                    lhsT = wT[:, half * 256 + ri * 128:
                              half * 256 + ri * 128 + 128]
                    r = half + ri     # V token-tile index
                    for eh in range(2):
                        nc.tensor.matmul(
                            ps_o[:, eh * 512:(eh + 1) * 512],
                            lhsT,
                            V[:, r, eh * 512:(eh + 1) * 512],
                            start=(ri == 0),
                            stop=(ri == 1),
                        )
                o_sb = outp.tile([128, D], F16, tag="o_sb")
                row0 = c * CH + half * 128
                if c < NCH - 1:
                    nc.scalar.activation(
                        o_sb[:, 0:512], ps_o[:, 0:512],
                        mybir.ActivationFunctionType.Copy, scale=recip,
                    )
                    nc.sync.dma_start(
                        out=out[row0:row0 + 128, 0:512], in_=o_sb[:, 0:512]
                    )
                    nc.vector.tensor_scalar_mul(
                        o_sb[:, 512:1024], ps_o[:, 512:1024], recip
                    )
                    nc.scalar.dma_start(
                        out=out[row0:row0 + 128, 512:1024],
                        in_=o_sb[:, 512:1024]
                    )
                else:
                    # last chunk: normalize+store in [128,256] quarters,
                    # alternating ACT/DVE and the two DGE queues, so the
                    # final epilogue chain is a quarter as deep
                    for qt in range(4):
                        qsl2 = slice(qt * 256, qt * 256 + 256)
                        if qt % 2 == 0:
                            nc.scalar.activation(
                                o_sb[:, qsl2], ps_o[:, qsl2],
                                mybir.ActivationFunctionType.Copy,
                                scale=recip,
                            )
                            nc.sync.dma_start(
                                out=out[row0:row0 + 128, qsl2],
                                in_=o_sb[:, qsl2]
                            )
                        else:
                            nc.vector.tensor_scalar_mul(
                                o_sb[:, qsl2], ps_o[:, qsl2], recip
                            )
                            nc.scalar.dma_start(
                                out=out[row0:row0 + 128, qsl2],
                                in_=o_sb[:, qsl2]
                            )
            xc_prev = xc

    _split_multi_waits(nc)
    return nc


_GRAPH = None


def _get_graph():
    global _GRAPH
    if _GRAPH is None:
        _GRAPH = _build_graph()
    return _GRAPH


# Cached jitted executable: run_bass_via_pjrt rebuilds its jit closure per
# call (a full retrace each time); replicate its multi-core path once and
# reuse it so repeated kernel() calls cost only the device execution.
_RUNNER = None


def _fast_run(nc, in_maps):
    global _RUNNER
    import jax
    from jax.experimental.shard_map import shard_map
    from jax.sharding import Mesh, PartitionSpec
    from concourse import bass2jax

    n_cores = len(in_maps)
    if _RUNNER is None:
        bass2jax.install_neuronx_cc_hook()
        partition_name = (
            nc.partition_id_tensor.name if nc.partition_id_tensor else None
        )
        in_names, out_names, out_avals = [], [], []
        for alloc in nc.m.functions[0].allocations:
            if not isinstance(alloc, mybir.MemoryLocationSet):
                continue
            name = alloc.memorylocations[0].name
            if alloc.kind == "ExternalInput":
                if name != partition_name:
                    in_names.append(name)
            elif alloc.kind == "ExternalOutput":
                out_names.append(name)
                out_avals.append(jax.core.ShapedArray(
                    tuple(alloc.tensor_shape), mybir.dt.np(alloc.dtype)))
        n_params = len(in_names)
        all_names = in_names + out_names
        if partition_name is not None:
            all_names.append(partition_name)
        donate = tuple(range(n_params, n_params + len(out_names)))

        def _body(*args):
            operands = list(args)
            if partition_name is not None:
                operands.append(bass2jax.partition_id_tensor())
            return tuple(bass2jax._bass_exec_p.bind(
                *operands,
                out_avals=tuple(out_avals),
                in_names=tuple(all_names),
                out_names=tuple(out_names),
                lowering_input_output_aliases=(),
                sim_require_finite=True,
                sim_require_nnan=True,
                nc=nc,
            ))

        devices = jax.devices()[:n_cores]
        mesh = Mesh(np.asarray(devices), ("core",))
        nio = n_params + len(out_names)
        sharded = jax.jit(
            shard_map(
                _body, mesh=mesh,
                in_specs=(PartitionSpec("core"),) * nio,
                out_specs=(PartitionSpec("core"),) * len(out_names),
                check_rep=False,
            ),
            donate_argnums=donate,
            keep_unused=True,
        )
        # allocate the donated output buffers on-device (no host transfer)
        import jax.numpy as jnp
        from jax.sharding import NamedSharding
        shardings = tuple(
            NamedSharding(mesh, PartitionSpec("core")) for _ in out_avals
        )
        zeros_jit = jax.jit(
            lambda: tuple(
                jnp.zeros((n_cores * av.shape[0], *av.shape[1:]), av.dtype)
                for av in out_avals
            ),
            out_shardings=shardings,
        )
        _RUNNER = (sharded, zeros_jit, in_names, out_names, out_avals)

    sharded, zeros_jit, in_names, out_names, out_avals = _RUNNER
    concat_in = [
        np.concatenate([np.asarray(m[name]) for m in in_maps], axis=0)
        for name in in_names
    ]
    out_arrs = sharded(*concat_in, *zeros_jit())
    return [
        {
            name: np.asarray(out_arrs[i]).reshape(
                n_cores, *out_avals[i].shape)[c]
            for i, name in enumerate(out_names)
        }
        for c in range(n_cores)
    ]


# ---------------------------------------------------------------------------
def _make_masks():
    """Additive masks in the packed 512-col layout (post-scale):
    [r0 vs qA | r1 vs qA | r1 vs qB | r2 vs qB], 128 q-cols each."""
    kp = np.arange(128)[:, None]
    qf = np.arange(128)[None, :]
    upper = np.where(kp > qf, 0.0, NEG)    # strict upper triangle kept
    lower = np.where(kp <= qf, 0.0, NEG)   # lower (incl diag) kept
    m = np.concatenate([upper, lower, upper, lower], axis=1).astype(np.float32)
    mf = m.copy()
    mf[:, 0:128] = NEG  # global block 0: the halo "previous block" is padding
    return m, mf


def kernel(x, Wq, bq, Wk, bk, Wv, bv, span):
    x = np.asarray(x)
    span_i = int(np.asarray(span))
    assert span_i == SPAN, f"kernel hardcodes span={SPAN}, got {span_i}"
    assert x.shape == (B, T, D)

    nc = _get_graph()

    Wq = np.asarray(Wq, dtype=np.float32)
    Wk = np.asarray(Wk, dtype=np.float32)
    # score matrix in x-space: s = x_kv M x_q^T with M = Wk^T Wq; the
    # kernel's z-chain lhsT wants mT[d2, d1] = M[d1, d2] = (Wq^T Wk)[d2, d1]
    mTh = np.ascontiguousarray(Wq.T @ Wk).astype(np.float16)
    bt = Wk.T @ np.asarray(bq, dtype=np.float32)          # [D], per-d1 bias
    bt8 = np.ascontiguousarray(bt.reshape(DT, 128).T).astype(
        np.float32, copy=False)
    wvT = np.ascontiguousarray(np.asarray(Wv).T).astype(np.float16)
    bvb = np.broadcast_to(np.asarray(bv), (128, D))
    m, mf = _make_masks()
    # packed [bv | maskc | maskf] -- one DMA instead of three
    cfc = np.ascontiguousarray(
        np.concatenate([bvb, m, m], axis=1)).astype(np.float32, copy=False)
    cff = np.ascontiguousarray(
        np.concatenate([bvb, m, mf], axis=1)).astype(np.float32, copy=False)
    ones = np.ones((128, 8), np.float16)

    in_maps = []
    for core in range(NCORES):
        b, h = divmod(core, 2)
        lo = h * TOKQ - HALO
        hi = (h + 1) * TOKQ
        xs = np.zeros((HALO + TOKQ, D), np.float32)
        if lo < 0:
            xs[HALO:] = x[b, 0:hi]
        else:
            xs[:] = x[b, lo:hi]
        # chunk-blocked x: [128, d, tok] per chunk block, d-major columns
        # (4-6KB contiguous per partition row per block -> fat DMA lines)
        xt3 = np.ascontiguousarray(
            xs.T.astype(np.float16).reshape(DT, 128, HALO + TOKQ)
            .transpose(1, 0, 2))          # [128, DT, HALO+TOKQ]
        blocks = [xt3[:, :, 0:KVW].reshape(128, DT * KVW)]
        for c in range(1, NCH):
            lo_c = HALO + c * CH
            blocks.append(
                xt3[:, :, lo_c:lo_c + CH].reshape(128, DT * CH))
        xch = np.ascontiguousarray(np.concatenate(blocks, axis=1))
        in_maps.append({
            "xch": xch, "mT": mTh, "wvT": wvT, "bt8": bt8,
            "constf": (cff if h == 0 else cfc), "onesp": ones,
        })

    if run_bass_kernel_spmd is _ORIG_RUN and not os.environ.get("BASS_TRACE"):
        results = _fast_run(nc, in_maps)
    else:
        # a harness monkeypatched run_bass_kernel_spmd (e.g. for tracing)
        results = run_bass_kernel_spmd(
            nc, in_maps, core_ids=list(range(NCORES))
        ).results

    out = np.empty((B, T, D), np.float32)
    for core in range(NCORES):
        b, h = divmod(core, 2)
        out[b, h * TOKQ:(h + 1) * TOKQ] = results[core]["out"].astype(
            np.float32
        )
    return out



# revision 22
# speedup vs baseline: 1.0269x; 1.0045x over previous
"""Trainium2 kernel for AdaptiveAttention (QKV projection + causal
sliding-window attention, span=128) on 8 NeuronCores.

Sharding: sequence-parallel with a 1-block halo -- 8 shards of
(batch b, sequence half h): each core owns 2048 query tokens and receives
128 halo tokens of x so attention needs no collectives (the window is
local).

Algorithmic restructure (vs the straightforward q = xWq^T+bq, k = xWk^T+bk,
s = q k^T): the score matrix only ever needs K through q k^T, so the host
precomputes M^T = Wq^T Wk and bt = Wk^T bq, and the kernel computes

    z[d1, q] = sum_d2 M^T[d2, d1] x_q[d2, q] + bt[d1]      (= "Q" in W-space)
    s[k, q]  = sum_d1 x_kv[d1, k] z[d1, q]                 (scores)

which equals q k^T + (terms constant in k). Constant-in-k terms cancel in
softmax, and the k-dependent bias cross-term folds exactly into bt. This
eliminates the entire K projection (a third of the projection FLOPs);
x_kv is just the raw input tile that is already in SBUF (the halo columns
come from the previous chunk's x tile, which ping-pongs).

Per-core layout strategy:
  - all matmuls in f16 operands (2^-11 rounding, below the accumulated
    matmul rounding); f32 PSUM accumulation.
  - x is passed pre-transposed AND pre-tiled per chunk ([128, d-major
    chunk blocks]) so every x DMA moves 128 lines of 4-6KB (the DGE is
    packet-rate-bound: 512B lines halve its effective bandwidth).
  - z is computed transposed (z^T in PSUM), drained with the bt bias add;
    scores sT[k, q] use x tiles as lhsT and z as rhs.
  - packed 512-col score layout (kv-tile r0 only vs the first 128-query
    half, r1 vs both, r2 only vs the second half), one PSUM bank per
    query half so each half's mask+exp only waits on its own chains;
    softmax denominators from a ones-matmul into the same PSUM tile.
  - mask+exp run per query half (DVE scale+mask, ACT exp); the output
    normalize is an ACT Copy-with-per-partition-scale straight out of
    PSUM, split in two so the two output-DMA halves go to the two
    hardware DGE queues (Sync + Scalar) in parallel.
  - input DMAs alternate between the two DGE queues (each descriptor
    submit costs ~620ns of engine time, and one queue sustains only
    ~280 GB/s); chunk-0's z/V projections run d-OUTER with all output
    tiles resident in PSUM so the PE starts on the first (x, M) d-tiles.
  - output is stored f16 and converted on the host.
"""

import sys

if "/opt/trn_rl_repo" not in sys.path:
    sys.path.insert(0, "/opt/trn_rl_repo")

import numpy as np
from contextlib import ExitStack

import os

import concourse.bass as bass
import concourse.mybir as mybir
import concourse.tile as tile
from concourse.bass_utils import run_bass_kernel_spmd
from concourse.vector_clock import ScopedClock

_ORIG_RUN = run_bass_kernel_spmd

# ---------------------------------------------------------------------------
# Problem constants (hardcoded per spec)
B, T, D = 4, 4096, 1024
SPAN = 128
NCORES = 8
TOKQ = T // 2           # 2048 query tokens per core
HALO = SPAN             # 128
CH = 256                # query chunk size (2 blocks = 1 block-pair)
NCH = TOKQ // CH        # 8 chunks
KVW = CH + HALO         # 384 KV tokens visible per chunk
DT = D // 128           # 8 d-tiles
NEG = -1.0e9
SCALE = 1.0 / 32.0      # 1/sqrt(D)

F32 = mybir.dt.float32
F32R = mybir.dt.float32r
F16 = mybir.dt.float16

# ---------------------------------------------------------------------------
# Walrus in this toolchain caps semaphore waits per instruction; Tile's
# kernel-tail Drain can exceed it. Chunk excess waits onto extra drains.
_MAX_WAITS = 1


def _patched_drain_and_barrier(self, tick_clock, wait_clock):
    nc = self.nc
    drain_inst = nc.sync.drain()
    wait_clock.add_sem_waits(
        drain_inst.ins, ScopedClock({None: tick_clock.global_clock})
    )
    si = drain_inst.ins.sync_info
    if si is not None and len(si.on_wait) > _MAX_WAITS:
        waits = list(si.on_wait)
        si.on_wait[:] = waits[:_MAX_WAITS]
        rest = waits[_MAX_WAITS:]
        while rest:
            extra = nc.sync.drain(fusable=False)
            extra.ins.sync_info = mybir.SyncInfo(
                on_wait=rest[:_MAX_WAITS], on_update=[]
            )
            rest = rest[_MAX_WAITS:]
    nc.all_engine_barrier()
    assert self.sems is not None
    popped = nc._tile_sem_poison_stack.pop()
    assert popped is self._sem_poison
    nc.clear_and_free_semaphores(list(self.sems.allocated().values()))
    nc.all_engine_barrier()


def _install_drain_patch():
    if getattr(tile.TileContext, "_drain_patch_installed", False):
        return
    tile.TileContext._drain_and_barrier = _patched_drain_and_barrier
    tile.TileContext._drain_patch_installed = True


def _split_multi_waits(nc, max_waits=_MAX_WAITS):
    """Walrus here supports one semaphore wait per instruction; hoist excess
    waits onto same-engine NoOps inserted immediately before."""
    for fn in nc.m.functions:
        for bb in fn.blocks:
            insts = bb.instructions
            out = []
            changed = False
            for inst in insts:
                si = getattr(inst, "sync_info", None)
                waits = list(si.on_wait) if si is not None else []
                if len(waits) > max_waits:
                    changed = True
                    for w in waits[:-max_waits]:
                        out.append(mybir.InstNoOp(
                            name=nc.get_next_instruction_name(),
                            sync_info=mybir.SyncInfo(on_wait=[w], on_update=[]),
                            bass_nofuse=True,
                            engine=inst.engine,
                        ))
                    si.on_wait[:] = waits[-max_waits:]
                out.append(inst)
            if changed:
                bb.instructions = out


# ---------------------------------------------------------------------------
def _build_graph():
    """Build the per-core Bass graph (SPMD: identical on all 8 cores)."""
    _install_drain_patch()
    nc = bass.Bass()

    # x, chunk-blocked: block 0 = [8 d-tiles x 384 cols] (halo+own of chunk
    # 0), blocks 1..7 = [8 x 256] (own tokens of chunks 1..7), concatenated
    # along the free dim; each 128-partition row is contiguous per block.
    XB0 = DT * KVW                  # 3072 cols in block 0
    XBC = DT * CH                   # 2048 cols per later block
    xch = nc.declare_dram_parameter(
        "xch", [128, XB0 + (NCH - 1) * XBC], F16, isOutput=False)
    mT = nc.declare_dram_parameter("mT", [D, D], F16, isOutput=False)
    wvT = nc.declare_dram_parameter("wvT", [D, D], F16, isOutput=False)
    bt8 = nc.declare_dram_parameter("bt8", [128, DT], F32, isOutput=False)
    # packed: constf = [bv | maskc | maskf]
    constf = nc.declare_dram_parameter("constf", [128, 2 * D], F32, isOutput=False)
    onesp = nc.declare_dram_parameter("onesp", [128, 8], F16, isOutput=False)
    out = nc.declare_dram_parameter("out", [TOKQ, D], F16, isOutput=True)

    with ExitStack() as ctx:
        tc = ctx.enter_context(tile.TileContext(nc))
        consts = ctx.enter_context(tc.tile_pool(name="consts", bufs=1))
        persist = ctx.enter_context(tc.tile_pool(name="persist", bufs=1))
        xc0p = ctx.enter_context(tc.tile_pool(name="xc0p", bufs=1))
        # bufs=3: chunk c's x tile stays alive through chunk c+1's scores
        # (halo); with only 2 bufs, chunk c+2's x DMA would wait for that
        # release and land ~3us after its z projection wants it.
        xcp = ctx.enter_context(tc.tile_pool(name="xcp", bufs=3))
        work = ctx.enter_context(tc.tile_pool(name="work", bufs=2))
        outp = ctx.enter_context(tc.tile_pool(name="outp", bufs=2))
        psp = ctx.enter_context(tc.tile_pool(name="psp", bufs=2, space="PSUM"))
        pss = ctx.enter_context(tc.tile_pool(name="pss", bufs=1, space="PSUM"))
        pso = ctx.enter_context(tc.tile_pool(name="pso", bufs=2, space="PSUM"))

        # --- warmup + resident weights/constants ------------------------
        warm = consts.tile([128, 128], F16, tag="warm")
        nc.vector.memset(warm, 0.0)
        # Score PSUM: one separate tile (= one bank) per 128-query half so
        # each half's mask+exp only waits on its own chains (Tile tracks
        # dependencies per tile); cols 256:264 hold the half's softmax
        # denominator. The warmup matmuls borrow ps_sA (free until the
        # chunk-0 scores).
        ps_sA = pss.tile([128, 512], F32, tag="sTA")
        ps_sB = pss.tile([128, 512], F32, tag="sTB")
        for _ in range(26):
            nc.tensor.matmul(ps_sA[:, 0:128], warm, warm, start=True, stop=True)

        # chunk-0 x in three submits (d0 alone so the first z matmuls start
        # earliest), M / Wv d-tiles alternating across the two DGE queues
        # in need-order; chunk-1's x on the lighter Scalar queue so its
        # hoisted z projection (below) can run before chunk-0's V.
        xc0 = xc0p.tile([128, DT, KVW], F16, tag="xc0")
        nc.sync.dma_start(out=xc0[:, 0, :], in_=xch[:, 0:KVW])
        mT_sb, wv_sb = [], []
        for d in range(DT):
            m = consts.tile([128, D], F16, tag=f"mT{d}")
            # cols 0:768 feed the chunk-0 d-OUTER phase (e0-e5); cols
            # 768:1024 only the later e6/e7 chains -- DMA'd separately
            # below so the just-in-time early stream is 25% lighter
            (nc.sync if d % 2 else nc.scalar).dma_start(
                out=m[:, 0:768], in_=mT[d * 128:(d + 1) * 128, 0:768])
            mT_sb.append(m)
            if d == 1:
                nc.sync.dma_start(
                    out=xc0[:, 1:4, :],
                    in_=xch[:, KVW:4 * KVW].rearrange("p (d c) -> p d c", d=3))
            if d == 2:
                nc.scalar.dma_start(
                    out=xc0[:, 4:8, :],
                    in_=xch[:, 4 * KVW:XB0].rearrange("p (d c) -> p d c", d=4))
        for d in range(DT):
            (nc.sync if d % 2 else nc.scalar).dma_start(
                out=mT_sb[d][:, 768:1024],
                in_=mT[d * 128:(d + 1) * 128, 768:1024])
        bt_sb = consts.tile([128, DT], F32, tag="bt8")
        nc.scalar.dma_start(out=bt_sb, in_=bt8[:, :])
        xc1 = xcp.tile([128, DT, CH], F16, tag="xc")
        nc.scalar.dma_start(
            out=xc1[:, :, :],
            in_=xch[:, XB0:XB0 + XBC].rearrange("p (d c) -> p d c", d=DT))
        for d in range(DT):
            wv = consts.tile([128, D], F16, tag=f"wv{d}")
            (nc.sync if d % 2 == 0 else nc.scalar).dma_start(
                out=wv, in_=wvT[d * 128:(d + 1) * 128, :])
            wv_sb.append(wv)
        cf_sb = consts.tile([128, 2 * D], F32, tag="constf")
        nc.scalar.dma_start(out=cf_sb, in_=constf[:, :])
        bv_sb = cf_sb[:, 0:D]
        mask_sb = cf_sb[:, D:D + 512]
        maskf_sb = cf_sb[:, D + 512:D + 1024]
        ones_sb = consts.tile([128, 8], F16, tag="ones")
        nc.scalar.dma_start(out=ones_sb, in_=onesp[:, :])

        # --- persistent per-chunk state (ping-pong) ---------------------
        # zT: [128, d1_tile, q], V: [128, tok_tile(3), e]
        zT2 = [persist.tile([128, DT, CH], F16, tag=f"zT{i}", name=f"zT{i}")
               for i in range(2)]
        V2 = [persist.tile([128, KVW // 128, D], F16, tag=f"V{i}", name=f"V{i}")
              for i in range(2)]

        xc_prev = None
        for c in range(NCH):
            zT, V = zT2[c % 2], V2[c % 2]
            Vp = V2[1 - c % 2]
            # ---- x chunk DMA (one submit, 4KB lines) -------------------
            if c == 0:
                xc = xc0
                own0 = HALO
                kv_t0 = 0
            elif c == 1:
                xc = xc1
                own0 = 0
                kv_t0 = 1
                nc.gpsimd.tensor_copy(V[:, 0, :], Vp[:, 2, :])
            else:
                xc = xcp.tile([128, DT, CH], F16, tag="xc")
                lo = XB0 + (c - 1) * XBC
                nc.sync.dma_start(
                    out=xc[:, :, :],
                    in_=xch[:, lo:lo + XBC].rearrange("p (d c) -> p d c", d=DT))
                own0 = 0
                kv_t0 = 1
                # previous chunk's last 128 V rows become this chunk's halo
                nc.gpsimd.tensor_copy(V[:, 0, :], Vp[:, 2, :])

            # k-tile lhsT sources for the score matmuls: (tile, col0) for
            # kv tiles 0 (halo), 1, 2 -- the halo columns live in the
            # previous chunk's x tile (ping-pong keeps it alive).
            if c == 0:
                ksrc = [(xc0, 0), (xc0, 128), (xc0, 256)]
            else:
                hal0 = KVW - HALO if c == 1 else CH - HALO
                ksrc = [(xc_prev, hal0), (xc, 0), (xc, 128)]

            # ---- z projection: zT = M^T-chain over d2, + bt bias -------
            def z_proj_eouter(xq, zT_):
                for e in range(DT):
                    ps = psp.tile([128, 512], F32, tag="proj")
                    for d in range(DT):
                        nc.tensor.matmul(
                            ps[:, 0:CH],
                            mT_sb[d][:, e * 128:(e + 1) * 128],
                            xq[:, d, 0:CH],
                            start=(d == 0),
                            stop=(d == DT - 1),
                        )
                    nc.vector.tensor_scalar_add(
                        zT_[:, e, :], ps[:, 0:CH], bt_sb[:, e:e + 1])

            if c == 0:
                # d-OUTER for e0-e5: accumulators resident in PSUM so the
                # first matmuls need only the first (x, M) d-tiles. ONE
                # accumulator per PSUM bank: a matmul with start=True
                # clears the whole destination bank's has_written bits, so
                # two interleaved accumulation chains must never share a
                # bank. e6/e7 run as ordinary chains after (all of M has
                # landed by then), leaving the score banks free for the
                # dummy matmuls that absorb DMA jitter between d-steps
                # (PE idle >3.4us here would re-throttle the HAM clock).
                psA = psp.tile([128, 512], F32, tag="proj")
                psB = psp.tile([128, 512], F32, tag="proj")
                psC = pso.tile([128, 1024], F32, tag="o")
                psD = pso.tile([128, 1024], F32, tag="o")
                zreg = [
                    (0, psA[:, 0:256]), (1, psB[:, 0:256]),
                    (2, psC[:, 0:256]), (3, psC[:, 512:768]),
                    (4, psD[:, 0:256]), (5, psD[:, 512:768]),
                ]
                for d in range(DT):
                    for e, reg in zreg:
                        nc.tensor.matmul(
                            reg,
                            mT_sb[d][:, e * 128:(e + 1) * 128],
                            xc[:, d, own0:own0 + CH],
                            start=(d == 0),
                            stop=(d == DT - 1),
                        )
                for e, reg in zreg:
                    nc.vector.tensor_scalar_add(
                        zT[:, e, :], reg, bt_sb[:, e:e + 1])
                for e, reg in ((6, ps_sA[:, 0:256]), (7, ps_sB[:, 0:256])):
                    for d in range(DT):
                        nc.tensor.matmul(
                            reg,
                            mT_sb[d][:, e * 128:(e + 1) * 128],
                            xc[:, d, own0:own0 + CH],
                            start=(d == 0),
                            stop=(d == DT - 1),
                        )
                    nc.vector.tensor_scalar_add(
                        zT[:, e, :], reg, bt_sb[:, e:e + 1])
                # chunk-1's z, software-pipelined here: it only needs the
                # resident M tiles + xc1 (landed long ago on the Scalar
                # queue), so the PE stays busy while Wv streams in for the
                # V projection below (no HAM re-throttle).
                z_proj_eouter(xc1, zT2[1])
            elif c > 1:
                z_proj_eouter(xc, zT)

            # ---- V projection (last psp user of the chunk: its drains
            # are long done when the next chunk's z wants the slots) -----
            if c == 0:
                # d-OUTER: 6 accumulators in pso+psp
                vs0 = pso.tile([128, 1024], F32, tag="o")
                vs1 = pso.tile([128, 1024], F32, tag="o")
                vs2 = psp.tile([128, 512], F32, tag="proj")
                vs3 = psp.tile([128, 512], F32, tag="proj")
                vreg = [
                    (0, 0, vs0[:, 0:512]), (0, 1, vs0[:, 512:1024]),
                    (1, 0, vs1[:, 0:512]), (1, 1, vs1[:, 512:1024]),
                    (2, 0, vs2), (2, 1, vs3),
                ]
                for d in range(DT):
                    for t, eh, reg in vreg:
                        nc.tensor.matmul(
                            reg,
                            xc[:, d, t * 128:(t + 1) * 128],
                            wv_sb[d][:, eh * 512:(eh + 1) * 512],
                            start=(d == 0),
                            stop=(d == DT - 1),
                        )
                for t, eh, reg in vreg:
                    nc.vector.tensor_add(
                        V[:, t, eh * 512:(eh + 1) * 512],
                        reg,
                        bv_sb[:, eh * 512:(eh + 1) * 512],
                    )
            else:
                for t in range(1, KVW // 128):
                    xcol = (t - 1) * 128
                    for eh in range(2):
                        ps = psp.tile([128, 512], F32, tag="proj")
                        for d in range(DT):
                            nc.tensor.matmul(
                                ps,
                                xc[:, d, xcol:xcol + 128],
                                wv_sb[d][:, eh * 512:(eh + 1) * 512],
                                start=(d == 0),
                                stop=(d == DT - 1),
                            )
                        nc.vector.tensor_add(
                            V[:, t, eh * 512:(eh + 1) * 512],
                            ps,
                            bv_sb[:, eh * 512:(eh + 1) * 512],
                        )

            # ---- attention -------------------------------------------
            # Packed score layout, one PSUM TILE (= one bank) per query
            # half: cols 0:128 prev-kv-tile scores, 128:256 own-kv-tile
            # scores, 256:264 the half's softmax denominator.
            ps_sA = pss.tile([128, 512], F32, tag="sTA")
            ps_sB = pss.tile([128, 512], F32, tag="sTB")
            schains = [
                (ps_sA, slice(0, 128), 0, slice(0, 128)),      # r0 qA
                (ps_sA, slice(128, 256), 1, slice(0, 128)),    # r1 qA
                (ps_sB, slice(0, 128), 1, slice(128, 256)),    # r1 qB
                (ps_sB, slice(128, 256), 2, slice(128, 256)),  # r2 qB
            ]
            for ps_sX, scol, kt, qsl in schains:
                lsrc, kc = ksrc[kt]
                for d1 in range(DT):
                    nc.tensor.matmul(
                        ps_sX[:, scol],
                        lsrc[:, d1, kc:kc + 128], zT[:, d1, qsl],
                        start=(d1 == 0), stop=(d1 == DT - 1),
                    )
            # wT = exp(sT * scale + mask), per query half (DVE then ACT)
            m_sb = maskf_sb if c == 0 else mask_sb
            stt = work.tile([128, 2 * CH], F32, tag="stt")
            wT = work.tile([128, 2 * CH], F16, tag="wT")
            for half, ps_sX in ((0, ps_sA), (1, ps_sB)):
                hsl = slice(half * 256, half * 256 + 256)
                nc.vector.scalar_tensor_tensor(
                    stt[:, hsl], ps_sX[:, 0:256], SCALE, m_sb[:, hsl],
                    mybir.AluOpType.mult, mybir.AluOpType.add,
                )
                nc.scalar.activation(
                    wT[:, hsl], stt[:, hsl], mybir.ActivationFunctionType.Exp
                )

            # Denominator matmuls FIRST so the reciprocal (DVE) overlaps
            # the o matmuls; normalize = Copy-with-scale straight from
            # PSUM (one e-half on ACT, one on DVE), out-DMA split across
            # the two DGE queues.
            for half, ps_sX in ((0, ps_sA), (1, ps_sB)):
                ps_o = pso.tile([128, 1024], F32, tag="o")
                for ri in range(2):
                    lhsT = wT[:, half * 256 + ri * 128:
                              half * 256 + ri * 128 + 128]
                    nc.tensor.matmul(
                        ps_sX[:, 256:264],
                        lhsT,
                        ones_sb,
                        start=(ri == 0),
                        stop=(ri == 1),
                    )
                recip = outp.tile([128, 1], F32, tag="recip")
                nc.vector.reciprocal(recip, ps_sX[:, 256:257])
                for ri in range(2):
                    lhsT = wT[:, half * 256 + ri * 128:
                              half * 256 + ri * 128 + 128]
                    r = half + ri     # V token-tile index
                    for eh in range(2):
                        nc.tensor.matmul(
                            ps_o[:, eh * 512:(eh + 1) * 512],
                            lhsT,
                            V[:, r, eh * 512:(eh + 1) * 512],
                            start=(ri == 0),
                            stop=(ri == 1),
                        )
                o_sb = outp.tile([128, D], F16, tag="o_sb")
                row0 = c * CH + half * 128
                nc.scalar.activation(
                    o_sb[:, 0:512], ps_o[:, 0:512],
                    mybir.ActivationFunctionType.Copy, scale=recip,
                )
                nc.sync.dma_start(
                    out=out[row0:row0 + 128, 0:512], in_=o_sb[:, 0:512]
                )
                nc.vector.tensor_scalar_mul(
                    o_sb[:, 512:1024], ps_o[:, 512:1024], recip
                )
                nc.scalar.dma_start(
                    out=out[row0:row0 + 128, 512:1024],
                    in_=o_sb[:, 512:1024]
                )
            xc_prev = xc

    _split_multi_waits(nc)
    return nc


_GRAPH = None


def _get_graph():
    global _GRAPH
    if _GRAPH is None:
        _GRAPH = _build_graph()
    return _GRAPH


# Cached jitted executable: run_bass_via_pjrt rebuilds its jit closure per
# call (a full retrace each time); replicate its multi-core path once and
# reuse it so repeated kernel() calls cost only the device execution.
_RUNNER = None


def _fast_run(nc, in_maps):
    global _RUNNER
    import jax
    from jax.experimental.shard_map import shard_map
    from jax.sharding import Mesh, PartitionSpec
    from concourse import bass2jax

    n_cores = len(in_maps)
    if _RUNNER is None:
        bass2jax.install_neuronx_cc_hook()
        partition_name = (
            nc.partition_id_tensor.name if nc.partition_id_tensor else None
        )
        in_names, out_names, out_avals = [], [], []
        for alloc in nc.m.functions[0].allocations:
            if not isinstance(alloc, mybir.MemoryLocationSet):
                continue
            name = alloc.memorylocations[0].name
            if alloc.kind == "ExternalInput":
                if name != partition_name:
                    in_names.append(name)
            elif alloc.kind == "ExternalOutput":
                out_names.append(name)
                out_avals.append(jax.core.ShapedArray(
                    tuple(alloc.tensor_shape), mybir.dt.np(alloc.dtype)))
        n_params = len(in_names)
        all_names = in_names + out_names
        if partition_name is not None:
            all_names.append(partition_name)
        donate = tuple(range(n_params, n_params + len(out_names)))

        def _body(*args):
            operands = list(args)
            if partition_name is not None:
                operands.append(bass2jax.partition_id_tensor())
            return tuple(bass2jax._bass_exec_p.bind(
                *operands,
                out_avals=tuple(out_avals),
                in_names=tuple(all_names),
                out_names=tuple(out_names),
                lowering_input_output_aliases=(),
                sim_require_finite=True,
                sim_require_nnan=True,
                nc=nc,
            ))

        devices = jax.devices()[:n_cores]
        mesh = Mesh(np.asarray(devices), ("core",))
        nio = n_params + len(out_names)
        sharded = jax.jit(
            shard_map(
                _body, mesh=mesh,
                in_specs=(PartitionSpec("core"),) * nio,
                out_specs=(PartitionSpec("core"),) * len(out_names),
                check_rep=False,
            ),
            donate_argnums=donate,
            keep_unused=True,
        )
        # allocate the donated output buffers on-device (no host transfer)
        import jax.numpy as jnp
        from jax.sharding import NamedSharding
        shardings = tuple(
            NamedSharding(mesh, PartitionSpec("core")) for _ in out_avals
        )
        zeros_jit = jax.jit(
            lambda: tuple(
                jnp.zeros((n_cores * av.shape[0], *av.shape[1:]), av.dtype)
                for av in out_avals
            ),
            out_shardings=shardings,
        )
        _RUNNER = (sharded, zeros_jit, in_names, out_names, out_avals)

    sharded, zeros_jit, in_names, out_names, out_avals = _RUNNER
    concat_in = [
        np.concatenate([np.asarray(m[name]) for m in in_maps], axis=0)
        for name in in_names
    ]
    out_arrs = sharded(*concat_in, *zeros_jit())
    return [
        {
            name: np.asarray(out_arrs[i]).reshape(
                n_cores, *out_avals[i].shape)[c]
            for i, name in enumerate(out_names)
        }
        for c in range(n_cores)
    ]


# ---------------------------------------------------------------------------
def _make_masks():
    """Additive masks in the packed 512-col layout (post-scale):
    [r0 vs qA | r1 vs qA | r1 vs qB | r2 vs qB], 128 q-cols each."""
    kp = np.arange(128)[:, None]
    qf = np.arange(128)[None, :]
    upper = np.where(kp > qf, 0.0, NEG)    # strict upper triangle kept
    lower = np.where(kp <= qf, 0.0, NEG)   # lower (incl diag) kept
    m = np.concatenate([upper, lower, upper, lower], axis=1).astype(np.float32)
    mf = m.copy()
    mf[:, 0:128] = NEG  # global block 0: the halo "previous block" is padding
    return m, mf


def kernel(x, Wq, bq, Wk, bk, Wv, bv, span):
    x = np.asarray(x)
    span_i = int(np.asarray(span))
    assert span_i == SPAN, f"kernel hardcodes span={SPAN}, got {span_i}"
    assert x.shape == (B, T, D)

    nc = _get_graph()

    Wq = np.asarray(Wq, dtype=np.float32)
    Wk = np.asarray(Wk, dtype=np.float32)
    # score matrix in x-space: s = x_kv M x_q^T with M = Wk^T Wq; the
    # kernel's z-chain lhsT wants mT[d2, d1] = M[d1, d2] = (Wq^T Wk)[d2, d1]
    mTh = np.ascontiguousarray(Wq.T @ Wk).astype(np.float16)
    bt = Wk.T @ np.asarray(bq, dtype=np.float32)          # [D], per-d1 bias
    bt8 = np.ascontiguousarray(bt.reshape(DT, 128).T).astype(
        np.float32, copy=False)
    wvT = np.ascontiguousarray(np.asarray(Wv).T).astype(np.float16)
    bvb = np.broadcast_to(np.asarray(bv), (128, D))
    m, mf = _make_masks()
    # packed [bv | maskc | maskf] -- one DMA instead of three
    cfc = np.ascontiguousarray(
        np.concatenate([bvb, m, m], axis=1)).astype(np.float32, copy=False)
    cff = np.ascontiguousarray(
        np.concatenate([bvb, m, mf], axis=1)).astype(np.float32, copy=False)
    ones = np.ones((128, 8), np.float16)

    in_maps = []
    for core in range(NCORES):
        b, h = divmod(core, 2)
        lo = h * TOKQ - HALO
        hi = (h + 1) * TOKQ
        xs = np.zeros((HALO + TOKQ, D), np.float32)
        if lo < 0:
            xs[HALO:] = x[b, 0:hi]
        else:
            xs[:] = x[b, lo:hi]
        # chunk-blocked x: [128, d, tok] per chunk block, d-major columns
        # (4-6KB contiguous per partition row per block -> fat DMA lines)
        xt3 = np.ascontiguousarray(
            xs.T.astype(np.float16).reshape(DT, 128, HALO + TOKQ)
            .transpose(1, 0, 2))          # [128, DT, HALO+TOKQ]
        blocks = [xt3[:, :, 0:KVW].reshape(128, DT * KVW)]
        for c in range(1, NCH):
            lo_c = HALO + c * CH
            blocks.append(
                xt3[:, :, lo_c:lo_c + CH].reshape(128, DT * CH))
        xch = np.ascontiguousarray(np.concatenate(blocks, axis=1))
        in_maps.append({
            "xch": xch, "mT": mTh, "wvT": wvT, "bt8": bt8,
            "constf": (cff if h == 0 else cfc), "onesp": ones,
        })

    if run_bass_kernel_spmd is _ORIG_RUN and not os.environ.get("BASS_TRACE"):
        results = _fast_run(nc, in_maps)
    else:
        # a harness monkeypatched run_bass_kernel_spmd (e.g. for tracing)
        results = run_bass_kernel_spmd(
            nc, in_maps, core_ids=list(range(NCORES))
        ).results

    out = np.empty((B, T, D), np.float32)
    for core in range(NCORES):
        b, h = divmod(core, 2)
        out[b, h * TOKQ:(h + 1) * TOKQ] = results[core]["out"].astype(
            np.float32
        )
    return out

